# revision 13
# baseline (speedup 1.0000x reference)
"""TRN2 Bass kernel v4 for nn_CrossAttention (B=32, C=512, 32x32 fmap, N=256 ctx).

Sharding: data-parallel over batch - 4 batches per core x 8 cores, weights
replicated.  All matmuls in bf16; softmax denominator via per-head-pair
ones-matmul broadcast; head-paired [128,1024] normalization; q-norm folded
into the bf16 fmap scaling multiply (2x-mode DVE); weights/ctxT/fmap
pre-converted to bf16 on the host; output returned in bf16.
Emission is software-pipelined across batches; all Sqrt activations cluster
in phase 0a (before any Exp) to avoid act-table reloads; phase-0a PSUM
tiles round-robin over three tags so evictions never stall the PE.
"""
import sys

sys.path.insert(0, "/opt/trn_rl_repo")
import itertools

import numpy as np
import ml_dtypes

BF16 = np.dtype(ml_dtypes.bfloat16)

B, C, X, Y = 32, 512, 32, 32
XY = X * Y
N, CCTX = 32 * 8, 768
H, D = 8, 64
DI = H * D  # 512
NCORES = 8
BPC = B // NCORES  # 4 batches per core

_cached = {}


def build_program():
    import concourse.bacc as bacc
    import concourse.mybir as mybir
    from concourse import tile

    f32 = mybir.dt.float32
    bf16 = mybir.dt.bfloat16
    Exp = mybir.ActivationFunctionType.Exp
    Sqrt = mybir.ActivationFunctionType.Sqrt

    nc = bacc.Bacc(num_devices=NCORES)

    fmap_d = nc.declare_dram_parameter("fmap", [BPC, C, XY], bf16, isOutput=False)
    ctx_d = nc.declare_dram_parameter("ctx", [BPC, N, CCTX], f32, isOutput=False)
    ctxT_d = nc.declare_dram_parameter("ctxT", [BPC, CCTX, N], bf16, isOutput=False)
    wqT_d = nc.declare_dram_parameter("wqT", [C, DI], bf16, isOutput=False)
    wkT_d = nc.declare_dram_parameter("wkT", [CCTX, DI], bf16, isOutput=False)
    wvT_d = nc.declare_dram_parameter("wvT", [CCTX, DI], bf16, isOutput=False)
    woT_d = nc.declare_dram_parameter("woT", [DI, C], bf16, isOutput=False)
    out_d = nc.declare_dram_parameter("out", [BPC, C, XY], bf16, isOutput=True)

    KC = C // 128      # 4 k-tiles over fmap channels
    KX = CCTX // 128   # 6 k-tiles over context channels
    MN = N // 128      # 2 key tiles
    MD = DI // 128     # 4 di tiles

    with tile.TileContext(nc) as tc:
        with (
            tc.tile_pool(name="wp", bufs=1) as wp,
            tc.tile_pool(name="io", bufs=1) as io,
            tc.tile_pool(name="ps", bufs=1, space="PSUM") as ps,
        ):
            PSB = {"big": 1, "sim": 2, "ot": 1}

            def ps_tile(tg, name):
                return ps.tile([128, 1024], mybir.dt.float32, tag=tg, bufs=PSB[tg], name=name)

            fmb = {}    # (b, k) bf16 fmap
            fmr = {}    # (b, k) bf16 scaled fmap
            cx2 = {}    # (pair, k) bf16 [128, 512] (2 batches)
            ctxn = {}
            s_ctx = {}  # (b, mk) [128,1]
            s_bc = {}   # b -> [128, XY] bf16
            kt2 = {}    # pair -> [128, 2048] bf16
            vaug = {}   # (b, mk) -> [128, 512] bf16
            qT = {}     # (b, m) -> [128, XY] bf16
            attnT = {}  # (b, tl) -> [128, XY] bf16
            fsq = {}    # (b, k) bf16 squares

            # ---- DMA emission helpers (ordered for fast pipeline fill) ----
            def load_w(dram, k, cols, tag):
                t = wp.tile([128, cols], bf16, tag=tag, name=f"w_{tag}")
                nc.sync.dma_start(out=t[:], in_=dram[k * 128:(k + 1) * 128, :])
                return t

            def load_ctxT(pair):
                p = pair[0] // 2
                for k in range(KX):
                    t = io.tile([128, 512], bf16, tag=f"c{k}", name=f"cx2_{p}_{k}")
                    for bi, b in enumerate(pair):
                        nc.sync.dma_start(out=t[:, bi * N:(bi + 1) * N],
                                          in_=ctxT_d[b, k * 128:(k + 1) * 128, :])
                    cx2[(p, k)] = t

            def load_ctx(pair):
                for b in pair:
                    for mk in range(MN):
                        t = io.tile([128, CCTX], f32, tag=f"n{mk}", bufs=2, name=f"ctxn{b}_{mk}")
                        nc.sync.dma_start(out=t[:], in_=ctx_d[b, mk * 128:(mk + 1) * 128, :])
                        ctxn[(b, mk)] = t

            def load_fmap(pair):
                for b in pair:
                    for k in range(KC):
                        t = io.tile([128, XY], bf16, tag=f"f{b}{k}", name=f"fmb_{b}_{k}")
                        nc.sync.dma_start(out=t[:], in_=fmap_d[b, k * 128:(k + 1) * 128, :])
                        fmb[(b, k)] = t

            def emit_sctx(b):
                # ctx sumsq: square on gpsimd (bf16 out), reduce + recip on DVE,
                # sqrt on scalar (clusters with the other phase-0 sqrts)
                for mk in range(MN):
                    cn = ctxn[(b, mk)]
                    scr = io.tile([128, CCTX], bf16, tag="scr", bufs=2, name=f"scr{b}_{mk}")
                    nc.gpsimd.tensor_mul(scr[:], cn[:], cn[:])
                    ssq = io.tile([128, 1], f32, tag=f"ssq{mk}", bufs=2, name=f"ssq{b}_{mk}")
                    nc.vector.reduce_sum(ssq[:], scr[:], axis=mybir.AxisListType.X)
                    rec = io.tile([128, 1], f32, tag=f"rec{mk}", bufs=2, name=f"rec{b}_{mk}")
                    nc.vector.reciprocal(rec[:], ssq[:])
                    sc = io.tile([128, 1], f32, tag=f"sc{b}{mk}", name=f"s_ctx{b}_{mk}")
                    nc.scalar.activation(sc[:], rec[:], Sqrt, scale=float(CCTX))
                    s_ctx[(b, mk)] = sc

            def emit_fsq(b, gate=None):
                # bf16 squares on DVE (2x mode); optional gate pins readiness
                MUL = mybir.AluOpType.mult
                for k in range(KC):
                    t = io.tile([128, XY], bf16, tag=f"s{k}", name=f"fsq{b}_{k}")
                    if gate is None:
                        nc.vector.tensor_mul(t[:], fmb[(b, k)][:], fmb[(b, k)][:])
                    else:
                        nc.vector.scalar_tensor_tensor(
                            t[:], fmb[(b, k)][:], gate[:, 0:1], fmb[(b, k)][:],
                            op0=MUL, op1=MUL)
                    fsq[(b, k)] = t

            # ---- unit generators (each unit = one PSUM-tile group) ----
            def units_kT(pair, tags=None):
                p = pair[0] // 2
                kt = io.tile([128, 2048], bf16, tag=f"kt{p}", name=f"kt2_{p}")
                kt2[p] = kt
                for mh in range(2):
                    def u(mh=mh, kt=kt, p=p):
                        tg = next(tags) if tags else "big"
                        pt = ps_tile(tg, f"pkT{p}{mh}")
                        for mi in range(2):
                            m = mh * 2 + mi
                            for k in range(KX):
                                nc.tensor.matmul(
                                    pt[:, mi * 512:(mi + 1) * 512],
                                    wk[k][:, m * 128:(m + 1) * 128], cx2[(p, k)][:],
                                    start=(k == 0), stop=(k == KX - 1))
                        nc.vector.tensor_copy(kt[:, mh * 1024:(mh + 1) * 1024], pt[:])
                    yield u

            def units_v(b, tags=None):
                p, bi = b // 2, b % 2
                def u():
                    tg = next(tags) if tags else "big"
                    pt = ps_tile(tg, f"pv{b}")
                    for mk in range(MN):
                        for k in range(KX):
                            nc.tensor.matmul(
                                pt[:, mk * 512:(mk + 1) * 512],
                                cx2[(p, k)][:, bi * N + mk * 128: bi * N + (mk + 1) * 128],
                                wv[k][:], start=(k == 0), stop=(k == KX - 1))
                    for mk in range(MN):
                        va = io.tile([128, DI], bf16, tag=f"v{b}{mk}", name=f"vaug{b}_{mk}")
                        vaug[(b, mk)] = va
                        nc.vector.tensor_scalar_mul(
                            va[:], pt[:, mk * 512:(mk + 1) * 512], s_ctx[(b, mk)][:])
                yield u

            def units_ss(b, tags=None):
                # fmap sumsq -> s_bcast (sqrt on scalar) -> bf16 scaled fmap (DVE 2x)
                def u():
                    tg = next(tags) if tags else "big"
                    pt = ps_tile(tg, f"pss{b}")
                    for k in range(KC):
                        for f in range(2):
                            nc.tensor.matmul(
                                pt[:, f * 512:(f + 1) * 512], ones_bf[:],
                                fsq[(b, k)][:, f * 512:(f + 1) * 512],
                                start=(k == 0), stop=(k == KC - 1))
                    recb = io.tile([128, XY], f32, tag="recb", name=f"recb{b}")
                    nc.vector.reciprocal_approx_fast(recb[:], pt[:])
                    sb = io.tile([128, XY], bf16, tag=f"sb{b % 2}", name=f"s_bc{b}")
                    nc.scalar.activation(sb[:], recb[:], Sqrt, scale=float(C) / float(D))
                    s_bc[b] = sb
                    for k in range(KC):
                        t = io.tile([128, XY], bf16, tag=f"r{b % 2}{k}", name=f"fmr{b}_{k}")
                        nc.vector.tensor_mul(t[:], fmb[(b, k)][:], sb[:])
                        fmr[(b, k)] = t
                yield u

            def units_qT(b, tags=None):
                for m in range(MD):
                    def u(m=m):
                        tg = next(tags) if tags else "big"
                        pt = ps_tile(tg, f"pq{b}{m}")
                        for k in range(KC):
                            for f in range(2):
                                nc.tensor.matmul(
                                    pt[:, f * 512:(f + 1) * 512],
                                    wq[k][:, m * 128:(m + 1) * 128],
                                    fmr[(b, k)][:, f * 512:(f + 1) * 512],
                                    start=(k == 0), stop=(k == KC - 1))
                        qt = io.tile([128, XY], bf16, tag=f"q{b % 2}{m}", name=f"qT{b}_{m}")
                        qT[(b, m)] = qt
                        nc.scalar.copy(qt[:], pt[:])
                    yield u

            def units_attn(b):
                p, bi = b // 2, b % 2
                for tl in range(MD):
                    at = io.tile([128, XY], bf16, tag=f"a{b % 2}{tl}", name=f"attnT{b}_{tl}")
                    attnT[(b, tl)] = at
                for hp in range(H // 2):   # head pairs; pair hp -> attnT tile hp
                    def u(hp=hp):
                        kt = kt2[p]
                        p_sb = {}
                        for hh in range(2):
                            h = 2 * hp + hh
                            ro = hh * D
                            for mk in range(MN):
                                st = ps_tile("sim", f"psim{b}{h}{mk}")
                                lhsT = kt[ro:ro + D,
                                          hp * 512 + bi * N + mk * 128: hp * 512 + bi * N + (mk + 1) * 128]
                                for f in range(2):
                                    nc.tensor.matmul(
                                        st[:, f * 512:(f + 1) * 512], lhsT,
                                        qT[(b, hp)][ro:ro + D, f * 512:(f + 1) * 512],
                                        start=True, stop=True)
                                pb = io.tile([128, XY], bf16, tag=f"p{hh}{mk}", bufs=2,
                                             name=f"p{b}{h}{mk}")
                                nc.scalar.activation(pb[:], st[:], Exp, scale=s_ctx[(b, mk)][:])
                                p_sb[(hh, mk)] = pb
                        # denominators for both heads -> [128, 1024] (rows 0-63 / 64-127)
                        dn = ps_tile("sim", f"pden{b}{hp}")
                        for hh in range(2):
                            for mk in range(MN):
                                for f in range(2):
                                    nc.tensor.matmul(
                                        dn[hh * D:(hh + 1) * D, f * 512:(f + 1) * 512],
                                        ones_bf[:, 0:D],
                                        p_sb[(hh, mk)][:, f * 512:(f + 1) * 512],
                                        start=(mk == 0), stop=(mk == MN - 1))
                        # attn @ v for both heads -> one [128, 1024] tile
                        ot = ps_tile("ot", f"pot{b}{hp}")
                        for hh in range(2):
                            h = 2 * hp + hh
                            for mk in range(MN):
                                for f in range(2):
                                    nc.tensor.matmul(
                                        ot[hh * D:(hh + 1) * D, f * 512:(f + 1) * 512],
                                        vaug[(b, mk)][:, h * D:(h + 1) * D],
                                        p_sb[(hh, mk)][:, f * 512:(f + 1) * 512],
                                        start=(mk == 0), stop=(mk == MN - 1))
                        db = io.tile([128, XY], f32, tag="dnb", bufs=2, name=f"dnb{b}{hp}")
                        nc.vector.reciprocal_approx_fast(db[:], dn[:])
                        nc.vector.tensor_mul(attnT[(b, hp)][:], ot[:], db[:])
                    yield u

            def units_out(b, tags=None):
                for m in range(KC):
                    def u(m=m):
                        tg = next(tags) if tags else "big"
                        pt = ps_tile(tg, f"po{b}{m}")
                        for k in range(MD):
                            for f in range(2):
                                nc.tensor.matmul(
                                    pt[:, f * 512:(f + 1) * 512],
                                    wo[k][:, m * 128:(m + 1) * 128],
                                    attnT[(b, k)][:, f * 512:(f + 1) * 512],
                                    start=(k == 0), stop=(k == MD - 1))
                        ob = io.tile([128, XY], bf16, tag="ob", bufs=2, name=f"ob{b}{m}")
                        nc.vector.tensor_copy(ob[:], pt[:])
                        nc.sync.dma_start(out=out_d[b, m * 128:(m + 1) * 128, :], in_=ob[:])
                    yield u

            def interleave(ga, gb):
                # proportional round-robin, B-stream slightly ahead
                la, lb = list(ga), list(gb)
                na, nb = len(la), len(lb)
                ia = ib = 0
                while ia < na or ib < nb:
                    fa = ia / na if na else 1.1
                    fb = ib / nb if nb else 1.1
                    if fb <= fa:
                        lb[ib]()
                        ib += 1
                    else:
                        la[ia]()
                        ia += 1

            def chain(*gens):
                for g in gens:
                    yield from g

            def run_all(g):
                for u in g:
                    u()

            P0, P1 = (0, 1), (2, 3)
            # ---- DMA emission in critical-path order ----
            wk = [load_w(wkT_d, k, DI, f"wk{k}") for k in range(KX)]
            load_ctxT(P0)
            load_ctx(P0)
            wv = [load_w(wvT_d, k, DI, f"wv{k}") for k in range(KX)]
            load_fmap(P0)
            wq = [load_w(wqT_d, k, DI, f"wq{k}") for k in range(KC)]
            wo = [load_w(woT_d, k, C, f"wo{k}") for k in range(MD)]
            ones_bf = wp.tile([128, 128], bf16, tag="ones", name="ones_bf")
            nc.vector.memset(ones_bf[:], 1.0)

            # ---- phase 0a: P0 projections + P1 norm prep (all sqrts here) ----
            rr = itertools.cycle(["big", "sim", "ot"])
            emit_sctx(0)
            emit_sctx(1)
            emit_fsq(0)
            emit_fsq(1)
            units0 = list(chain(units_kT(P0, rr), units_v(0, rr), units_v(1, rr),
                                units_ss(0, rr), units_ss(1, rr), units_qT(0, rr)))
            for u in units0[:4]:
                u()
            load_fmap(P1)
            load_ctxT(P1)
            load_ctx(P1)
            emit_sctx(2)
            emit_sctx(3)
            units0[4]()  # ss(0)
            gate = io.tile([128, 1], f32, tag="gate", name="fsq_gate")
            nc.vector.memset(gate[:], 1.0)
            for u in units0[5:]:
                u()
            emit_fsq(2, gate=gate)
            emit_fsq(3, gate=gate)
            run_all(chain(units_ss(2, rr), units_ss(3, rr)))
            # ---- phase 0b: attn(0) x qT(1) ----
            interleave(units_attn(0), units_qT(1))
            # ---- phases 1-4 fused: continuous interleave, no phase seams ----
            ro2 = itertools.cycle(["big", "ot"])
            interleave(chain(units_attn(1), units_attn(2), units_attn(3)),
                       chain(units_kT(P1), units_v(2), units_v(3), units_qT(2),
                             units_out(0), units_qT(3), units_out(1),
                             units_out(2)))
            run_all(units_out(3, ro2))

    nc.compile()
    return nc


def _prep_inputs(fmap, context, mask, gamma_fmap, gamma_ctx, Wq, Wkv, Wout):
    fmap = np.asarray(fmap, dtype=np.float32).reshape(B, C, XY).astype(BF16)
    context = np.ascontiguousarray(np.asarray(context, dtype=np.float32))
    ctxT = np.ascontiguousarray(context.transpose(0, 2, 1).astype(BF16))
    gf = np.asarray(gamma_fmap, dtype=np.float32)
    gc = np.asarray(gamma_ctx, dtype=np.float32)
    wqT = np.ascontiguousarray((np.asarray(Wq, np.float32) * gf[None, :]).T.astype(BF16))
    wkT = np.ascontiguousarray((np.asarray(Wkv, np.float32)[:DI] * gc[None, :]).T.astype(BF16))
    wvT = np.ascontiguousarray((np.asarray(Wkv, np.float32)[DI:] * gc[None, :]).T.astype(BF16))
    woT = np.ascontiguousarray(np.asarray(Wout, np.float32).T.astype(BF16))
    in_maps = []
    for c in range(NCORES):
        sl = slice(c * BPC, (c + 1) * BPC)
        in_maps.append({
            "fmap": np.ascontiguousarray(fmap[sl]),
            "ctx": np.ascontiguousarray(context[sl]),
            "ctxT": np.ascontiguousarray(ctxT[sl]),
            "wqT": wqT, "wkT": wkT, "wvT": wvT, "woT": woT,
        })
    return in_maps


def run(trace=False, **inputs):
    from concourse.bass_utils import run_bass_kernel_spmd

    if "nc" not in _cached:
        _cached["nc"] = build_program()
    nc = _cached["nc"]
    in_maps = _prep_inputs(**inputs)
    try:
        res = run_bass_kernel_spmd(nc, in_maps, list(range(NCORES)), trace=trace)
    except ModuleNotFoundError:
        res = run_bass_kernel_spmd(nc, in_maps, list(range(NCORES)), trace=False)
    out = np.empty((B, C, X, Y), dtype=np.float32)
    for c in range(NCORES):
        out[c * BPC:(c + 1) * BPC] = (
            np.asarray(res.results[c]["out"]).astype(np.float32).reshape(BPC, C, X, Y))
    return out, res.exec_time_ns


def kernel(**inputs):
    out, _ = run(trace=False, **inputs)
    return out


# revision 15
# speedup vs baseline: 1.0176x; 1.0176x over previous
"""TRN2 Bass kernel v4 for nn_CrossAttention (B=32, C=512, 32x32 fmap, N=256 ctx).

Sharding: data-parallel over batch - 4 batches per core x 8 cores, weights
replicated.  All matmuls in bf16; softmax denominator via per-head-pair
ones-matmul broadcast; head-paired [128,1024] normalization; q-norm folded
into the bf16 fmap scaling multiply (2x-mode DVE); weights/ctxT/fmap
pre-converted to bf16 on the host; output returned in bf16.
Emission is software-pipelined across batches; all Sqrt activations cluster
in phase 0a (before any Exp) to avoid act-table reloads; phase-0a PSUM
tiles round-robin over three tags so evictions never stall the PE.
"""
import sys

sys.path.insert(0, "/opt/trn_rl_repo")
import itertools

import numpy as np
import ml_dtypes

BF16 = np.dtype(ml_dtypes.bfloat16)

B, C, X, Y = 32, 512, 32, 32
XY = X * Y
N, CCTX = 32 * 8, 768
H, D = 8, 64
DI = H * D  # 512
NCORES = 8
BPC = B // NCORES  # 4 batches per core

_cached = {}


def build_program():
    import concourse.bacc as bacc
    import concourse.mybir as mybir
    from concourse import tile

    f32 = mybir.dt.float32
    bf16 = mybir.dt.bfloat16
    Exp = mybir.ActivationFunctionType.Exp
    Sqrt = mybir.ActivationFunctionType.Sqrt

    nc = bacc.Bacc(num_devices=NCORES)

    fmap_d = nc.declare_dram_parameter("fmap", [BPC, C, XY], bf16, isOutput=False)
    ctx_d = nc.declare_dram_parameter("ctx", [BPC, N, CCTX], f32, isOutput=False)
    ctxT_d = nc.declare_dram_parameter("ctxT", [BPC, CCTX, N], bf16, isOutput=False)
    wqT_d = nc.declare_dram_parameter("wqT", [C, DI], bf16, isOutput=False)
    wkT_d = nc.declare_dram_parameter("wkT", [CCTX, DI], bf16, isOutput=False)
    wvT_d = nc.declare_dram_parameter("wvT", [CCTX, DI], bf16, isOutput=False)
    woT_d = nc.declare_dram_parameter("woT", [DI, C], bf16, isOutput=False)
    out_d = nc.declare_dram_parameter("out", [BPC, C, XY], bf16, isOutput=True)

    KC = C // 128      # 4 k-tiles over fmap channels
    KX = CCTX // 128   # 6 k-tiles over context channels
    MN = N // 128      # 2 key tiles
    MD = DI // 128     # 4 di tiles

    with tile.TileContext(nc) as tc:
        with (
            tc.tile_pool(name="wp", bufs=1) as wp,
            tc.tile_pool(name="io", bufs=1) as io,
            tc.tile_pool(name="ps", bufs=1, space="PSUM") as ps,
        ):
            PSB = {"big": 1, "sim": 2, "ot": 1}

            def ps_tile(tg, name):
                return ps.tile([128, 1024], mybir.dt.float32, tag=tg, bufs=PSB[tg], name=name)

            fmb = {}    # (b, k) bf16 fmap
            fmr = {}    # (b, k) bf16 scaled fmap
            cx2 = {}    # (pair, k) bf16 [128, 512] (2 batches)
            ctxn = {}
            s_ctx = {}  # (b, mk) [128,1]
            s_bc = {}   # b -> [128, XY] bf16
            kt2 = {}    # pair -> [128, 2048] bf16
            vaug = {}   # (b, mk) -> [128, 512] bf16
            qT = {}     # (b, m) -> [128, XY] bf16
            attnT = {}  # (b, tl) -> [128, XY] bf16
            fsq = {}    # (b, k) bf16 squares

            # ---- DMA emission helpers (ordered for fast pipeline fill) ----
            def load_w(dram, k, cols, tag):
                t = wp.tile([128, cols], bf16, tag=tag, name=f"w_{tag}")
                nc.sync.dma_start(out=t[:], in_=dram[k * 128:(k + 1) * 128, :])
                return t

            def load_ctxT(pair):
                p = pair[0] // 2
                for k in range(KX):
                    t = io.tile([128, 512], bf16, tag=f"c{k}", name=f"cx2_{p}_{k}")
                    for bi, b in enumerate(pair):
                        nc.sync.dma_start(out=t[:, bi * N:(bi + 1) * N],
                                          in_=ctxT_d[b, k * 128:(k + 1) * 128, :])
                    cx2[(p, k)] = t

            def load_ctx(pair):
                for b in pair:
                    for mk in range(MN):
                        t = io.tile([128, CCTX], f32, tag=f"n{mk}", bufs=2, name=f"ctxn{b}_{mk}")
                        nc.sync.dma_start(out=t[:], in_=ctx_d[b, mk * 128:(mk + 1) * 128, :])
                        ctxn[(b, mk)] = t

            def load_fmap(pair):
                for b in pair:
                    for k in range(KC):
                        t = io.tile([128, XY], bf16, tag=f"f{b}{k}", name=f"fmb_{b}_{k}")
                        nc.sync.dma_start(out=t[:], in_=fmap_d[b, k * 128:(k + 1) * 128, :])
                        fmb[(b, k)] = t

            def emit_sctx(b):
                # ctx sumsq: square on gpsimd (bf16 out), reduce + recip on DVE,
                # sqrt on scalar (clusters with the other phase-0 sqrts)
                for mk in range(MN):
                    cn = ctxn[(b, mk)]
                    scr = io.tile([128, CCTX], bf16, tag="scr", bufs=2, name=f"scr{b}_{mk}")
                    nc.gpsimd.tensor_mul(scr[:], cn[:], cn[:])
                    ssq = io.tile([128, 1], f32, tag=f"ssq{mk}", bufs=2, name=f"ssq{b}_{mk}")
                    nc.vector.reduce_sum(ssq[:], scr[:], axis=mybir.AxisListType.X)
                    rec = io.tile([128, 1], f32, tag=f"rec{mk}", bufs=2, name=f"rec{b}_{mk}")
                    nc.vector.reciprocal(rec[:], ssq[:])
                    sc = io.tile([128, 1], f32, tag=f"sc{b}{mk}", name=f"s_ctx{b}_{mk}")
                    nc.scalar.activation(sc[:], rec[:], Sqrt, scale=float(CCTX))
                    s_ctx[(b, mk)] = sc

            def emit_fsq(b, gate=None):
                # bf16 squares on DVE (2x mode); optional gate pins readiness
                MUL = mybir.AluOpType.mult
                for k in range(KC):
                    t = io.tile([128, XY], bf16, tag=f"s{k}", name=f"fsq{b}_{k}")
                    if gate is None:
                        nc.vector.tensor_mul(t[:], fmb[(b, k)][:], fmb[(b, k)][:])
                    else:
                        nc.vector.scalar_tensor_tensor(
                            t[:], fmb[(b, k)][:], gate[:, 0:1], fmb[(b, k)][:],
                            op0=MUL, op1=MUL)
                    fsq[(b, k)] = t

            # ---- unit generators (each unit = one PSUM-tile group) ----
            def units_kT(pair, tags=None):
                p = pair[0] // 2
                kt = io.tile([128, 2048], bf16, tag=f"kt{p}", name=f"kt2_{p}")
                kt2[p] = kt
                for mh in range(2):
                    def u(mh=mh, kt=kt, p=p):
                        tg = next(tags) if tags else "big"
                        pt = ps_tile(tg, f"pkT{p}{mh}")
                        for mi in range(2):
                            m = mh * 2 + mi
                            for k in range(KX):
                                nc.tensor.matmul(
                                    pt[:, mi * 512:(mi + 1) * 512],
                                    wk[k][:, m * 128:(m + 1) * 128], cx2[(p, k)][:],
                                    start=(k == 0), stop=(k == KX - 1))
                        nc.vector.tensor_copy(kt[:, mh * 1024:(mh + 1) * 1024], pt[:])
                    yield u

            def units_v(b, tags=None):
                p, bi = b // 2, b % 2
                def u():
                    tg = next(tags) if tags else "big"
                    pt = ps_tile(tg, f"pv{b}")
                    for mk in range(MN):
                        for k in range(KX):
                            nc.tensor.matmul(
                                pt[:, mk * 512:(mk + 1) * 512],
                                cx2[(p, k)][:, bi * N + mk * 128: bi * N + (mk + 1) * 128],
                                wv[k][:], start=(k == 0), stop=(k == KX - 1))
                    for mk in range(MN):
                        va = io.tile([128, DI], bf16, tag=f"v{b}{mk}", name=f"vaug{b}_{mk}")
                        vaug[(b, mk)] = va
                        nc.vector.tensor_scalar_mul(
                            va[:], pt[:, mk * 512:(mk + 1) * 512], s_ctx[(b, mk)][:])
                yield u

            def units_ss(b, tags=None):
                # fmap sumsq -> s_bcast (sqrt on scalar) -> bf16 scaled fmap (DVE 2x)
                def u():
                    tg = next(tags) if tags else "big"
                    pt = ps_tile(tg, f"pss{b}")
                    for k in range(KC):
                        for f in range(2):
                            nc.tensor.matmul(
                                pt[:, f * 512:(f + 1) * 512], ones_bf[:],
                                fsq[(b, k)][:, f * 512:(f + 1) * 512],
                                start=(k == 0), stop=(k == KC - 1))
                    recb = io.tile([128, XY], f32, tag="recb", name=f"recb{b}")
                    nc.vector.reciprocal_approx_fast(recb[:], pt[:])
                    sb = io.tile([128, XY], bf16, tag=f"sb{b % 2}", name=f"s_bc{b}")
                    nc.scalar.activation(sb[:], recb[:], Sqrt, scale=float(C) / float(D))
                    s_bc[b] = sb
                    for k in range(KC):
                        t = io.tile([128, XY], bf16, tag=f"r{b % 2}{k}", name=f"fmr{b}_{k}")
                        nc.vector.tensor_mul(t[:], fmb[(b, k)][:], sb[:])
                        fmr[(b, k)] = t
                yield u

            def units_qT(b, tags=None):
                for m in range(MD):
                    def u(m=m):
                        tg = next(tags) if tags else "big"
                        pt = ps_tile(tg, f"pq{b}{m}")
                        for k in range(KC):
                            for f in range(2):
                                nc.tensor.matmul(
                                    pt[:, f * 512:(f + 1) * 512],
                                    wq[k][:, m * 128:(m + 1) * 128],
                                    fmr[(b, k)][:, f * 512:(f + 1) * 512],
                                    start=(k == 0), stop=(k == KC - 1))
                        qt = io.tile([128, XY], bf16, tag=f"q{b % 2}{m}", name=f"qT{b}_{m}")
                        qT[(b, m)] = qt
                        nc.scalar.copy(qt[:], pt[:])
                    yield u

            def units_attn(b):
                p, bi = b // 2, b % 2
                for tl in range(MD):
                    at = io.tile([128, XY], bf16, tag=f"a{b % 2}{tl}", name=f"attnT{b}_{tl}")
                    attnT[(b, tl)] = at
                for hp in range(H // 2):   # head pairs; pair hp -> attnT tile hp
                    def u(hp=hp):
                        kt = kt2[p]
                        p_sb = {}
                        for hh in range(2):
                            h = 2 * hp + hh
                            ro = hh * D
                            for mk in range(MN):
                                st = ps_tile("sim", f"psim{b}{h}{mk}")
                                lhsT = kt[ro:ro + D,
                                          hp * 512 + bi * N + mk * 128: hp * 512 + bi * N + (mk + 1) * 128]
                                for f in range(2):
                                    nc.tensor.matmul(
                                        st[:, f * 512:(f + 1) * 512], lhsT,
                                        qT[(b, hp)][ro:ro + D, f * 512:(f + 1) * 512],
                                        start=True, stop=True)
                                pb = io.tile([128, XY], bf16, tag=f"p{hh}{mk}", bufs=2,
                                             name=f"p{b}{h}{mk}")
                                nc.scalar.activation(pb[:], st[:], Exp, scale=s_ctx[(b, mk)][:])
                                p_sb[(hh, mk)] = pb
                        # denominators for both heads -> [128, 1024] (rows 0-63 / 64-127)
                        dn = ps_tile("sim", f"pden{b}{hp}")
                        for hh in range(2):
                            for mk in range(MN):
                                for f in range(2):
                                    nc.tensor.matmul(
                                        dn[hh * D:(hh + 1) * D, f * 512:(f + 1) * 512],
                                        ones_bf[:, 0:D],
                                        p_sb[(hh, mk)][:, f * 512:(f + 1) * 512],
                                        start=(mk == 0), stop=(mk == MN - 1))
                        # attn @ v for both heads -> one [128, 1024] tile
                        ot = ps_tile("ot", f"pot{b}{hp}")
                        for hh in range(2):
                            h = 2 * hp + hh
                            for mk in range(MN):
                                for f in range(2):
                                    nc.tensor.matmul(
                                        ot[hh * D:(hh + 1) * D, f * 512:(f + 1) * 512],
                                        vaug[(b, mk)][:, h * D:(h + 1) * D],
                                        p_sb[(hh, mk)][:, f * 512:(f + 1) * 512],
                                        start=(mk == 0), stop=(mk == MN - 1))
                        db = io.tile([128, XY], f32, tag="dnb", bufs=2, name=f"dnb{b}{hp}")
                        nc.vector.reciprocal_approx_fast(db[:], dn[:])
                        nc.vector.tensor_mul(attnT[(b, hp)][:], ot[:], db[:])
                    yield u

            def units_out(b, tags=None):
                for m in range(KC):
                    def u(m=m):
                        tg = next(tags) if tags else "big"
                        pt = ps_tile(tg, f"po{b}{m}")
                        for k in range(MD):
                            for f in range(2):
                                nc.tensor.matmul(
                                    pt[:, f * 512:(f + 1) * 512],
                                    wo[k][:, m * 128:(m + 1) * 128],
                                    attnT[(b, k)][:, f * 512:(f + 1) * 512],
                                    start=(k == 0), stop=(k == MD - 1))
                        ob = io.tile([128, XY], bf16, tag="ob", bufs=2, name=f"ob{b}{m}")
                        nc.vector.tensor_copy(ob[:], pt[:])
                        nc.sync.dma_start(out=out_d[b, m * 128:(m + 1) * 128, :], in_=ob[:])
                    yield u

            def interleave(ga, gb):
                # proportional round-robin, B-stream slightly ahead
                la, lb = list(ga), list(gb)
                na, nb = len(la), len(lb)
                ia = ib = 0
                while ia < na or ib < nb:
                    fa = ia / na if na else 1.1
                    fb = ib / nb if nb else 1.1
                    if fb <= fa:
                        lb[ib]()
                        ib += 1
                    else:
                        la[ia]()
                        ia += 1

            def chain(*gens):
                for g in gens:
                    yield from g

            def run_all(g):
                for u in g:
                    u()

            P0, P1 = (0, 1), (2, 3)
            # ---- DMA emission in critical-path order ----
            wk = [load_w(wkT_d, k, DI, f"wk{k}") for k in range(KX)]
            load_ctxT(P0)
            load_ctx(P0)
            wv = [load_w(wvT_d, k, DI, f"wv{k}") for k in range(KX)]
            load_fmap(P0)
            wq = [load_w(wqT_d, k, DI, f"wq{k}") for k in range(KC)]
            wo = [load_w(woT_d, k, C, f"wo{k}") for k in range(MD)]
            ones_bf = wp.tile([128, 128], bf16, tag="ones", name="ones_bf")
            nc.vector.memset(ones_bf[:], 1.0)

            # ---- phase 0a: P0 projections + P1 norm prep (all sqrts here) ----
            rr = itertools.cycle(["big", "sim", "ot"])
            emit_sctx(0)
            emit_sctx(1)
            emit_fsq(0)
            emit_fsq(1)
            units0 = list(chain(units_kT(P0, rr), units_v(0, rr), units_v(1, rr),
                                units_ss(0, rr), units_ss(1, rr)))
            for u in units0[:4]:
                u()
            load_fmap(P1)
            load_ctxT(P1)
            load_ctx(P1)
            emit_sctx(2)
            emit_sctx(3)
            units0[4]()  # ss(0)
            gate = io.tile([128, 1], f32, tag="gate", name="fsq_gate")
            nc.vector.memset(gate[:], 1.0)
            units0[5]()  # ss(1)
            emit_fsq(2, gate=gate)
            emit_fsq(3, gate=gate)
            # ---- phase 0b: attn(0) x (qT(0), ss(2), ss(3)) ----
            interleave(units_attn(0),
                       chain(units_qT(0), units_ss(2), units_ss(3)))
            # ---- phase 1: attn(1) x (qT(1), kT(P1), v(2), v(3), qT(2)) ----
            interleave(units_attn(1),
                       chain(units_qT(1), units_kT(P1), units_v(2), units_v(3),
                             units_qT(2)))
            # ---- phase 2: attn(2) x (out(0), qT(3)) ----
            interleave(units_attn(2), chain(units_out(0), units_qT(3)))
            # ---- phase 3: attn(3) x out(1) ----
            interleave(units_attn(3), units_out(1))
            # ---- phase 4: out(2), out(3) (alternate big/ot psum tags) ----
            ro2 = itertools.cycle(["big", "ot"])
            run_all(chain(units_out(2, ro2), units_out(3, ro2)))

    nc.compile()
    return nc


def _prep_inputs(fmap, context, mask, gamma_fmap, gamma_ctx, Wq, Wkv, Wout):
    fmap = np.asarray(fmap, dtype=np.float32).reshape(B, C, XY).astype(BF16)
    context = np.ascontiguousarray(np.asarray(context, dtype=np.float32))
    ctxT = np.ascontiguousarray(context.transpose(0, 2, 1).astype(BF16))
    gf = np.asarray(gamma_fmap, dtype=np.float32)
    gc = np.asarray(gamma_ctx, dtype=np.float32)
    wqT = np.ascontiguousarray((np.asarray(Wq, np.float32) * gf[None, :]).T.astype(BF16))
    wkT = np.ascontiguousarray((np.asarray(Wkv, np.float32)[:DI] * gc[None, :]).T.astype(BF16))
    wvT = np.ascontiguousarray((np.asarray(Wkv, np.float32)[DI:] * gc[None, :]).T.astype(BF16))
    woT = np.ascontiguousarray(np.asarray(Wout, np.float32).T.astype(BF16))
    in_maps = []
    for c in range(NCORES):
        sl = slice(c * BPC, (c + 1) * BPC)
        in_maps.append({
            "fmap": np.ascontiguousarray(fmap[sl]),
            "ctx": np.ascontiguousarray(context[sl]),
            "ctxT": np.ascontiguousarray(ctxT[sl]),
            "wqT": wqT, "wkT": wkT, "wvT": wvT, "woT": woT,
        })
    return in_maps


def run(trace=False, **inputs):
    from concourse.bass_utils import run_bass_kernel_spmd

    if "nc" not in _cached:
        _cached["nc"] = build_program()
    nc = _cached["nc"]
    in_maps = _prep_inputs(**inputs)
    try:
        res = run_bass_kernel_spmd(nc, in_maps, list(range(NCORES)), trace=trace)
    except ModuleNotFoundError:
        res = run_bass_kernel_spmd(nc, in_maps, list(range(NCORES)), trace=False)
    out = np.empty((B, C, X, Y), dtype=np.float32)
    for c in range(NCORES):
        out[c * BPC:(c + 1) * BPC] = (
            np.asarray(res.results[c]["out"]).astype(np.float32).reshape(BPC, C, X, Y))
    return out, res.exec_time_ns


def kernel(**inputs):
    out, _ = run(trace=False, **inputs)
    return out


# revision 17
# speedup vs baseline: 1.0280x; 1.0102x over previous
"""TRN2 Bass kernel v4 for nn_CrossAttention (B=32, C=512, 32x32 fmap, N=256 ctx).

Sharding: data-parallel over batch - 4 batches per core x 8 cores, weights
replicated.  All matmuls in bf16; softmax denominator via per-head-pair
ones-matmul broadcast; head-paired [128,1024] normalization; q-norm folded
into the bf16 fmap scaling multiply (2x-mode DVE); weights/ctxT/fmap
pre-converted to bf16 on the host; output returned in bf16.
Emission is software-pipelined across batches; all Sqrt activations cluster
in phase 0a (before any Exp) to avoid act-table reloads; phase-0a PSUM
tiles round-robin over three tags so evictions never stall the PE.
"""
import sys

sys.path.insert(0, "/opt/trn_rl_repo")
import itertools

import numpy as np
import ml_dtypes

BF16 = np.dtype(ml_dtypes.bfloat16)

B, C, X, Y = 32, 512, 32, 32
XY = X * Y
N, CCTX = 32 * 8, 768
H, D = 8, 64
DI = H * D  # 512
NCORES = 8
BPC = B // NCORES  # 4 batches per core

_cached = {}


def build_program():
    import concourse.bacc as bacc
    import concourse.mybir as mybir
    from concourse import tile

    f32 = mybir.dt.float32
    bf16 = mybir.dt.bfloat16
    Exp = mybir.ActivationFunctionType.Exp
    Sqrt = mybir.ActivationFunctionType.Sqrt

    nc = bacc.Bacc(num_devices=NCORES)

    fmap_d = nc.declare_dram_parameter("fmap", [BPC, C, XY], bf16, isOutput=False)
    ctx_d = nc.declare_dram_parameter("ctx", [BPC, N, CCTX], f32, isOutput=False)
    ctxT_d = nc.declare_dram_parameter("ctxT", [BPC, CCTX, N], bf16, isOutput=False)
    wqT_d = nc.declare_dram_parameter("wqT", [C, DI], bf16, isOutput=False)
    wkT_d = nc.declare_dram_parameter("wkT", [CCTX, DI], bf16, isOutput=False)
    wvT_d = nc.declare_dram_parameter("wvT", [CCTX, DI], bf16, isOutput=False)
    woT_d = nc.declare_dram_parameter("woT", [DI, C], bf16, isOutput=False)
    out_d = nc.declare_dram_parameter("out", [BPC, C, XY], bf16, isOutput=True)

    KC = C // 128      # 4 k-tiles over fmap channels
    KX = CCTX // 128   # 6 k-tiles over context channels
    MN = N // 128      # 2 key tiles
    MD = DI // 128     # 4 di tiles

    with tile.TileContext(nc) as tc:
        with (
            tc.tile_pool(name="wp", bufs=1) as wp,
            tc.tile_pool(name="io", bufs=1) as io,
            tc.tile_pool(name="ps", bufs=1, space="PSUM") as ps,
        ):
            PSB = {"big": 1, "sim": 2, "ot": 1}

            def ps_tile(tg, name):
                return ps.tile([128, 1024], mybir.dt.float32, tag=tg, bufs=PSB[tg], name=name)

            fmb = {}    # (b, k) bf16 fmap
            fmr = {}    # (b, k) bf16 scaled fmap
            cx2 = {}    # (pair, k) bf16 [128, 512] (2 batches)
            ctxn = {}
            s_ctx = {}  # (b, mk) [128,1]
            s_bc = {}   # b -> [128, XY] bf16
            kt2 = {}    # pair -> [128, 2048] bf16
            vaug = {}   # (b, mk) -> [128, 512] bf16
            qT = {}     # (b, m) -> [128, XY] bf16
            attnT = {}  # (b, tl) -> [128, XY] bf16
            fsq = {}    # (b, k) bf16 squares

            # ---- DMA emission helpers (ordered for fast pipeline fill) ----
            def load_w(dram, k, cols, tag):
                t = wp.tile([128, cols], bf16, tag=tag, name=f"w_{tag}")
                nc.sync.dma_start(out=t[:], in_=dram[k * 128:(k + 1) * 128, :])
                return t

            def load_ctxT(pair):
                p = pair[0] // 2
                for k in range(KX):
                    t = io.tile([128, 512], bf16, tag=f"c{k}", name=f"cx2_{p}_{k}")
                    for bi, b in enumerate(pair):
                        nc.sync.dma_start(out=t[:, bi * N:(bi + 1) * N],
                                          in_=ctxT_d[b, k * 128:(k + 1) * 128, :])
                    cx2[(p, k)] = t

            def load_ctx(pair):
                for b in pair:
                    for mk in range(MN):
                        t = io.tile([128, CCTX], f32, tag=f"n{mk}", bufs=2, name=f"ctxn{b}_{mk}")
                        nc.sync.dma_start(out=t[:], in_=ctx_d[b, mk * 128:(mk + 1) * 128, :])
                        ctxn[(b, mk)] = t

            def load_fmap(pair):
                for b in pair:
                    for k in range(KC):
                        t = io.tile([128, XY], bf16, tag=f"f{b}{k}", name=f"fmb_{b}_{k}")
                        nc.sync.dma_start(out=t[:], in_=fmap_d[b, k * 128:(k + 1) * 128, :])
                        fmb[(b, k)] = t

            def emit_sctx(b):
                # ctx sumsq: square on gpsimd (bf16 out), reduce + recip on DVE,
                # sqrt on scalar (clusters with the other phase-0 sqrts)
                for mk in range(MN):
                    cn = ctxn[(b, mk)]
                    scr = io.tile([128, CCTX], bf16, tag="scr", bufs=2, name=f"scr{b}_{mk}")
                    nc.gpsimd.tensor_mul(scr[:], cn[:], cn[:])
                    ssq = io.tile([128, 1], f32, tag=f"ssq{mk}", bufs=2, name=f"ssq{b}_{mk}")
                    nc.vector.reduce_sum(ssq[:], scr[:], axis=mybir.AxisListType.X)
                    rec = io.tile([128, 1], f32, tag=f"rec{mk}", bufs=2, name=f"rec{b}_{mk}")
                    nc.vector.reciprocal(rec[:], ssq[:])
                    sc = io.tile([128, 1], f32, tag=f"sc{b}{mk}", name=f"s_ctx{b}_{mk}")
                    nc.scalar.activation(sc[:], rec[:], Sqrt, scale=float(CCTX))
                    s_ctx[(b, mk)] = sc

            def emit_fsq(b, gate=None):
                # bf16 squares on DVE (2x mode); optional gate pins readiness
                MUL = mybir.AluOpType.mult
                for k in range(KC):
                    t = io.tile([128, XY], bf16, tag=f"s{k}", name=f"fsq{b}_{k}")
                    if gate is None:
                        nc.vector.tensor_mul(t[:], fmb[(b, k)][:], fmb[(b, k)][:])
                    else:
                        nc.vector.scalar_tensor_tensor(
                            t[:], fmb[(b, k)][:], gate[:, 0:1], fmb[(b, k)][:],
                            op0=MUL, op1=MUL)
                    fsq[(b, k)] = t

            # ---- unit generators (each unit = one PSUM-tile group) ----
            def units_kT(pair, tags=None):
                p = pair[0] // 2
                kt = io.tile([128, 2048], bf16, tag=f"kt{p}", name=f"kt2_{p}")
                kt2[p] = kt
                for mh in range(2):
                    def u(mh=mh, kt=kt, p=p):
                        tg = next(tags) if tags else "big"
                        pt = ps_tile(tg, f"pkT{p}{mh}")
                        for mi in range(2):
                            m = mh * 2 + mi
                            for k in range(KX):
                                nc.tensor.matmul(
                                    pt[:, mi * 512:(mi + 1) * 512],
                                    wk[k][:, m * 128:(m + 1) * 128], cx2[(p, k)][:],
                                    start=(k == 0), stop=(k == KX - 1))
                        nc.vector.tensor_copy(kt[:, mh * 1024:(mh + 1) * 1024], pt[:])
                    yield u

            def units_v(b, tags=None):
                p, bi = b // 2, b % 2
                def u():
                    tg = next(tags) if tags else "big"
                    pt = ps_tile(tg, f"pv{b}")
                    for mk in range(MN):
                        for k in range(KX):
                            nc.tensor.matmul(
                                pt[:, mk * 512:(mk + 1) * 512],
                                cx2[(p, k)][:, bi * N + mk * 128: bi * N + (mk + 1) * 128],
                                wv[k][:], start=(k == 0), stop=(k == KX - 1))
                    for mk in range(MN):
                        va = io.tile([128, DI], bf16, tag=f"v{b}{mk}", name=f"vaug{b}_{mk}")
                        vaug[(b, mk)] = va
                        nc.vector.tensor_scalar_mul(
                            va[:], pt[:, mk * 512:(mk + 1) * 512], s_ctx[(b, mk)][:])
                yield u

            def units_ss(b, tags=None):
                # fmap sumsq -> s_bcast (sqrt on scalar) -> bf16 scaled fmap (DVE 2x)
                def u():
                    tg = next(tags) if tags else "big"
                    pt = ps_tile(tg, f"pss{b}")
                    for k in range(KC):
                        for f in range(2):
                            nc.tensor.matmul(
                                pt[:, f * 512:(f + 1) * 512], ones_bf[:],
                                fsq[(b, k)][:, f * 512:(f + 1) * 512],
                                start=(k == 0), stop=(k == KC - 1))
                    recb = io.tile([128, XY], f32, tag="recb", name=f"recb{b}")
                    nc.vector.reciprocal_approx_fast(recb[:], pt[:])
                    sb = io.tile([128, XY], bf16, tag=f"sb{b % 2}", name=f"s_bc{b}")
                    nc.scalar.activation(sb[:], recb[:], Sqrt, scale=float(C) / float(D))
                    s_bc[b] = sb
                    for k in range(KC):
                        t = io.tile([128, XY], bf16, tag=f"r{b % 2}{k}", name=f"fmr{b}_{k}")
                        nc.vector.tensor_mul(t[:], fmb[(b, k)][:], sb[:])
                        fmr[(b, k)] = t
                yield u

            def units_qT(b, tags=None):
                for m in range(MD):
                    def u(m=m):
                        tg = next(tags) if tags else "big"
                        pt = ps_tile(tg, f"pq{b}{m}")
                        for k in range(KC):
                            for f in range(2):
                                nc.tensor.matmul(
                                    pt[:, f * 512:(f + 1) * 512],
                                    wq[k][:, m * 128:(m + 1) * 128],
                                    fmr[(b, k)][:, f * 512:(f + 1) * 512],
                                    start=(k == 0), stop=(k == KC - 1))
                        qt = io.tile([128, XY], bf16, tag=f"q{b % 2}{m}", name=f"qT{b}_{m}")
                        qT[(b, m)] = qt
                        nc.scalar.copy(qt[:], pt[:])
                    yield u

            def units_attn(b):
                p, bi = b // 2, b % 2
                for tl in range(MD):
                    at = io.tile([128, XY], bf16, tag=f"a{b % 2}{tl}", name=f"attnT{b}_{tl}")
                    attnT[(b, tl)] = at
                for hp in range(H // 2):   # head pairs; pair hp -> attnT tile hp
                    def u(hp=hp):
                        kt = kt2[p]
                        p_sb = {}
                        for hh in range(2):
                            h = 2 * hp + hh
                            ro = hh * D
                            for mk in range(MN):
                                st = ps_tile("sim", f"psim{b}{h}{mk}")
                                lhsT = kt[ro:ro + D,
                                          hp * 512 + bi * N + mk * 128: hp * 512 + bi * N + (mk + 1) * 128]
                                for f in range(2):
                                    nc.tensor.matmul(
                                        st[:, f * 512:(f + 1) * 512], lhsT,
                                        qT[(b, hp)][ro:ro + D, f * 512:(f + 1) * 512],
                                        start=True, stop=True)
                                pb = io.tile([128, XY], bf16, tag=f"p{hh}{mk}", bufs=2,
                                             name=f"p{b}{h}{mk}")
                                nc.scalar.activation(pb[:], st[:], Exp, scale=s_ctx[(b, mk)][:])
                                p_sb[(hh, mk)] = pb
                        # denominators for both heads -> [128, 1024] (rows 0-63 / 64-127)
                        dn = ps_tile("sim", f"pden{b}{hp}")
                        for hh in range(2):
                            for mk in range(MN):
                                for f in range(2):
                                    nc.tensor.matmul(
                                        dn[hh * D:(hh + 1) * D, f * 512:(f + 1) * 512],
                                        ones_bf[:, 0:D],
                                        p_sb[(hh, mk)][:, f * 512:(f + 1) * 512],
                                        start=(mk == 0), stop=(mk == MN - 1))
                        # attn @ v for both heads -> one [128, 1024] tile
                        ot = ps_tile("ot", f"pot{b}{hp}")
                        for hh in range(2):
                            h = 2 * hp + hh
                            for mk in range(MN):
                                for f in range(2):
                                    nc.tensor.matmul(
                                        ot[hh * D:(hh + 1) * D, f * 512:(f + 1) * 512],
                                        vaug[(b, mk)][:, h * D:(h + 1) * D],
                                        p_sb[(hh, mk)][:, f * 512:(f + 1) * 512],
                                        start=(mk == 0), stop=(mk == MN - 1))
                        db = io.tile([128, XY], f32, tag="dnb", bufs=2, name=f"dnb{b}{hp}")
                        nc.vector.reciprocal_approx_fast(db[:], dn[:])
                        nc.vector.tensor_mul(attnT[(b, hp)][:], ot[:], db[:])
                    yield u

            def units_out(b, tags=None):
                for m in range(KC):
                    def u(m=m):
                        tg = next(tags) if tags else "big"
                        pt = ps_tile(tg, f"po{b}{m}")
                        for k in range(MD):
                            for f in range(2):
                                nc.tensor.matmul(
                                    pt[:, f * 512:(f + 1) * 512],
                                    wo[k][:, m * 128:(m + 1) * 128],
                                    attnT[(b, k)][:, f * 512:(f + 1) * 512],
                                    start=(k == 0), stop=(k == MD - 1))
                        ob = io.tile([128, XY], bf16, tag="ob", bufs=2, name=f"ob{b}{m}")
                        nc.vector.tensor_copy(ob[:], pt[:])
                        nc.sync.dma_start(out=out_d[b, m * 128:(m + 1) * 128, :], in_=ob[:])
                    yield u

            def interleave(ga, gb):
                # proportional round-robin, B-stream slightly ahead
                la, lb = list(ga), list(gb)
                na, nb = len(la), len(lb)
                ia = ib = 0
                while ia < na or ib < nb:
                    fa = ia / na if na else 1.1
                    fb = ib / nb if nb else 1.1
                    if fb <= fa:
                        lb[ib]()
                        ib += 1
                    else:
                        la[ia]()
                        ia += 1

            def chain(*gens):
                for g in gens:
                    yield from g

            def run_all(g):
                for u in g:
                    u()

            P0, P1 = (0, 1), (2, 3)
            # ---- DMA emission in critical-path order ----
            wk = [load_w(wkT_d, k, DI, f"wk{k}") for k in range(KX)]
            load_ctxT(P0)
            load_ctx(P0)
            wv = [load_w(wvT_d, k, DI, f"wv{k}") for k in range(KX)]
            load_fmap(P0)
            wq = [load_w(wqT_d, k, DI, f"wq{k}") for k in range(KC)]
            wo = [load_w(woT_d, k, C, f"wo{k}") for k in range(MD)]
            ones_bf = wp.tile([128, 128], bf16, tag="ones", name="ones_bf")
            nc.vector.memset(ones_bf[:], 1.0)

            # ---- phase 0a: P0 projections + P1 norm prep (all sqrts here) ----
            rr = itertools.cycle(["big", "sim", "ot"])
            emit_sctx(0)
            emit_sctx(1)
            emit_fsq(0)
            emit_fsq(1)
            units0 = list(chain(units_kT(P0, rr), units_v(0, rr), units_v(1, rr),
                                units_ss(0, rr), units_ss(1, rr)))
            for u in units0[:4]:
                u()
            load_fmap(P1)
            load_ctxT(P1)
            load_ctx(P1)
            emit_sctx(2)
            emit_sctx(3)
            units0[4]()  # ss(0)
            gate = io.tile([128, 1], f32, tag="gate", name="fsq_gate")
            nc.vector.memset(gate[:], 1.0)
            units0[5]()  # ss(1)
            emit_fsq(2, gate=gate)
            emit_fsq(3, gate=gate)
            # ---- phase 0b: attn(0) x (qT(0), ss(2), ss(3)) ----
            interleave(units_attn(0),
                       chain(units_qT(0), units_ss(2), units_ss(3)))
            # ---- phase 1: attn(1) x (qT(1), kT(P1), v(2), v(3), qT(2)) ----
            interleave(units_attn(1),
                       chain(units_qT(1), units_kT(P1), units_v(2), units_v(3),
                             units_qT(2)))
            # ---- phase 2: attn(2) x (out(0), qT(3)) ----
            interleave(units_attn(2), chain(units_out(0), units_qT(3)))
            # ---- phase 3: attn(3) x out(1) ----
            interleave(units_attn(3), units_out(1))
            # ---- phase 4: out(2), out(3) (alternate big/ot psum tags) ----
            ro2 = itertools.cycle(["big", "ot"])
            run_all(chain(units_out(2, ro2), units_out(3, ro2)))

    nc.compile()
    return nc


def _prep_inputs(fmap, context, mask, gamma_fmap, gamma_ctx, Wq, Wkv, Wout):
    fmap = np.asarray(fmap, dtype=np.float32).reshape(B, C, XY).astype(BF16)
    context = np.ascontiguousarray(np.asarray(context, dtype=np.float32))
    ctxT = np.ascontiguousarray(context.transpose(0, 2, 1).astype(BF16))
    gf = np.asarray(gamma_fmap, dtype=np.float32)
    gc = np.asarray(gamma_ctx, dtype=np.float32)
    wqT = np.ascontiguousarray((np.asarray(Wq, np.float32) * gf[None, :]).T.astype(BF16))
    wkT = np.ascontiguousarray((np.asarray(Wkv, np.float32)[:DI] * gc[None, :]).T.astype(BF16))
    wvT = np.ascontiguousarray((np.asarray(Wkv, np.float32)[DI:] * gc[None, :]).T.astype(BF16))
    woT = np.ascontiguousarray(np.asarray(Wout, np.float32).T.astype(BF16))
    in_maps = []
    for c in range(NCORES):
        sl = slice(c * BPC, (c + 1) * BPC)
        in_maps.append({
            "fmap": np.ascontiguousarray(fmap[sl]),
            "ctx": np.ascontiguousarray(context[sl]),
            "ctxT": np.ascontiguousarray(ctxT[sl]),
            "wqT": wqT, "wkT": wkT, "wvT": wvT, "woT": woT,
        })
    return in_maps


def run(trace=False, **inputs):
    from concourse.bass_utils import run_bass_kernel_spmd

    if "nc" not in _cached:
        _cached["nc"] = build_program()
    nc = _cached["nc"]
    in_maps = _prep_inputs(**inputs)
    try:
        res = run_bass_kernel_spmd(nc, in_maps, list(range(NCORES)), trace=trace)
    except ModuleNotFoundError:
        res = run_bass_kernel_spmd(nc, in_maps, list(range(NCORES)), trace=False)
    out = np.empty((B, C, X, Y), dtype=np.float32)
    for c in range(NCORES):
        out[c * BPC:(c + 1) * BPC] = (
            np.asarray(res.results[c]["out"]).astype(np.float32).reshape(BPC, C, X, Y))
    return out, res.exec_time_ns


def kernel(**inputs):
    out, _ = run(trace=False, **inputs)
    return out
